# revision 1
# baseline (speedup 1.0000x reference)
"""Trainium2 Bass kernel for ExtensibleAttention (sparse_attention).

Strategy: data-parallel over the 65536 tokens (N*L flattened) across 8
NeuronCores; the small 256-dim projection weights are replicated. All
per-token math is fused into one pass per 512-token tile:

  q/k/v/pos projections as PE matmuls in [C, T] layout (channel on
  partitions, token on free dim), with q+pos / k+pos fused into the PSUM
  accumulation; offset MLP (relu + second projection) likewise; the
  grid-sample weight w, softmax over K=4 sample points, and the final
  out-projection all on-chip.

Inputs are pre-transposed to [C, T] on the host (numpy) so the kernel
needs no on-chip transposes: matmul contracts over the partition dim, so
activations must be channel-major anyway. Head reductions (sum over d
within a head), the k-broadcast of qk, the sum over K, and the
head->channel broadcast of wv are done as matmuls against small constant
0/1 matrices. The Wo2 columns are host-permuted from (h,k,c) to (c,h,k)
order so the x/y coordinates occupy partition halves, making the
grid-sample weight product a single partition-offset vector multiply.
"""

import numpy as np
from contextlib import ExitStack

import concourse.bacc as bacc
import concourse.tile as tile
from concourse import mybir

F32 = mybir.dt.float32
F32R = mybir.dt.float32r
AF = mybir.ActivationFunctionType

N, L, C, H, KP, D = 4, 16384, 256, 8, 4, 32
NCORES = 8
TOKS = N * L // NCORES  # 8192 tokens per core
TLOAD = 512             # tokens per DMA load tile
TCOMP = 512             # tokens per compute tile (PSUM free-dim limit, fp32)
SIGMA = float(1.0 / np.sqrt(D))


def _build(toks=TOKS, tload=TLOAD, with_bias=False):
    nc = bacc.Bacc(trn_type="TRN2")
    dram = {}

    def din(name, shape, dt=None):
        dram[name] = nc.dram_tensor(name, list(shape), dt or F32R,
                                    kind="ExternalInput")
        return dram[name]

    xq = din("xq", (128, 2, toks))
    xk = din("xk", (128, 2, toks))
    xv = din("xv", (128, 2, toks))
    xp = din("xp", (128, 2, toks))
    ref = din("ref", (2, toks))
    din("wq", (128, 2, 256))
    din("wk", (128, 2, 256))
    din("wv", (128, 2, 256))
    din("wp", (128, 2, 256))
    din("wo1", (128, 2, 512))
    din("wo2", (128, 4, 64))
    din("wo", (128, 2, 256))
    din("bo1", (128, 4))
    din("bwof", (64, 1))
    din("smat", (64, 32))
    din("amat", (128, 64))
    din("cmat", (32, 8))
    din("bmat", (8, 256))
    din("pmat", (2, 64))
    if with_bias:
        din("ones", (1, 512))
        din("bqp", (1, 256))
        din("bkp", (1, 256))
        din("bvr", (1, 256))
        din("bor", (1, 256))
    out = nc.dram_tensor("out", [toks, 256], F32, kind="ExternalOutput")

    nload = toks // tload
    nsub = tload // TCOMP
    T = TCOMP

    with tile.TileContext(nc) as tc, ExitStack() as ctx:
        singles = ctx.enter_context(tc.tile_pool(name="singles", bufs=1))
        inp = ctx.enter_context(tc.tile_pool(name="inp", bufs=4))
        work = ctx.enter_context(tc.tile_pool(name="work", bufs=2))
        psA = ctx.enter_context(tc.tile_pool(name="psA", bufs=3, space="PSUM"))
        psB = ctx.enter_context(tc.tile_pool(name="psB", bufs=5, space="PSUM"))

        def load1(name, shape, dt=F32R):
            t = singles.tile(list(shape), dt, name=f"sb_{name}")
            nc.sync.dma_start(out=t, in_=dram[name][:])
            return t

        mm = nc.tensor.matmul

        def load_tile(lt):
            t0 = lt * tload
            xv_t = inp.tile([128, 2, tload], F32R, tag="xv")
            nc.sync.dma_start(out=xv_t, in_=xv[:, :, t0:t0 + tload])
            xq_t = inp.tile([128, 2, tload], F32R, tag="xq")
            nc.sync.dma_start(out=xq_t, in_=xq[:, :, t0:t0 + tload])
            xp_t = inp.tile([128, 2, tload], F32R, tag="xp")
            nc.sync.dma_start(out=xp_t, in_=xp[:, :, t0:t0 + tload])
            xk_t = inp.tile([128, 2, tload], F32R, tag="xk")
            nc.sync.dma_start(out=xk_t, in_=xk[:, :, t0:t0 + tload])
            ref_t = inp.tile([2, tload], F32R, tag="ref")
            nc.sync.dma_start(out=ref_t, in_=ref[:, t0:t0 + tload])
            return xq_t, xp_t, xk_t, xv_t, ref_t

        def stage1(ld, lo, tz):
            """Projection matmuls + q*k product + hidden/offset MLP."""
            xq_t, xp_t, xk_t, xv_t, ref_t = ld
            s = slice(lo, lo + tz)

            # v = value@Wv  (per-chunk 1-bank PSUM tiles: slot reuse only
            # depends on ACT copies of the previous tile, never on DVE)
            v_sb = work.tile([128, 2, tz], F32, tag="v", bufs=3)
            for mc in range(2):
                m128 = slice(mc * 128, (mc + 1) * 128)
                v_ps = psA.tile([128, tz], F32, tag="bigA")
                mm(v_ps, wv_s[:, 0, m128], xv_t[:, 0, s], start=True, stop=False)
                mm(v_ps, wv_s[:, 1, m128], xv_t[:, 1, s], start=False,
                   stop=not with_bias)
                if with_bias:
                    mm(v_ps, bvr_s[:, m128], ones_s[:, :tz], start=False, stop=True)
                nc.scalar.copy(v_sb[:, mc, :], v_ps)

            # q/k projections (+pos fused into the PSUM accumulation) and the
            # q*k product, one 128-channel chunk at a time so each chunk's
            # PSUM bank frees while the next chunk's matmuls run
            q_sb = work.tile([128, 2, tz], F32, tag="qsb", bufs=1)
            k_sb = work.tile([128, 2, tz], F32, tag="ksb", bufs=1)
            m_sb = work.tile([128, 2, tz], F32R, tag="m", bufs=2)
            for mc in range(2):
                m128 = slice(mc * 128, (mc + 1) * 128)
                q_ps = psA.tile([128, tz], F32, tag="bigA")
                mm(q_ps, wq_s[:, 0, m128], xq_t[:, 0, s], start=True, stop=False)
                mm(q_ps, wq_s[:, 1, m128], xq_t[:, 1, s], start=False, stop=False)
                mm(q_ps, wp_s[:, 0, m128], xp_t[:, 0, s], start=False, stop=False)
                mm(q_ps, wp_s[:, 1, m128], xp_t[:, 1, s], start=False,
                   stop=not with_bias)
                if with_bias:
                    mm(q_ps, bqp_s[:, m128], ones_s[:, :tz], start=False, stop=True)
                k_ps = psA.tile([128, tz], F32, tag="bigA")
                mm(k_ps, wk_s[:, 0, m128], xk_t[:, 0, s], start=True, stop=False)
                mm(k_ps, wk_s[:, 1, m128], xk_t[:, 1, s], start=False, stop=False)
                mm(k_ps, wp_s[:, 0, m128], xp_t[:, 0, s], start=False, stop=False)
                mm(k_ps, wp_s[:, 1, m128], xp_t[:, 1, s], start=False,
                   stop=not with_bias)
                if with_bias:
                    mm(k_ps, bkp_s[:, m128], ones_s[:, :tz], start=False, stop=True)
                # ACT copies release the PSUM banks immediately; the q*k
                # product runs on the otherwise-idle GPSIMD (SBUF-only)
                nc.scalar.copy(q_sb[:, mc, :], q_ps)
                nc.scalar.copy(k_sb[:, mc, :], k_ps)
                nc.gpsimd.tensor_mul(m_sb[:, mc, :], q_sb[:, mc, :],
                                     k_sb[:, mc, :])
            # hidden = relu(query@Wo1 + bo1), 4 chunks of 128
            hid_sb = work.tile([128, 4, tz], F32R, tag="hid", bufs=1)
            for j in range(4):
                h_ps = psB.tile([128, tz], F32, tag="small")
                j128 = slice(j * 128, (j + 1) * 128)
                mm(h_ps, wo1_s[:, 0, j128], xq_t[:, 0, s], start=True, stop=False)
                mm(h_ps, wo1_s[:, 1, j128], xq_t[:, 1, s], start=False, stop=True)
                nc.scalar.activation(hid_sb[:, j, :], h_ps, AF.Relu,
                                     bias=bo1_s[:, j:j + 1], scale=1.0)

            # off = hidden@Wo2p + ref, rows = (c,h,k) with x coords in
            # partitions 0-31 and y coords in 32-63
            off_ps = psB.tile([64, tz], F32, tag="small")
            for j in range(4):
                mm(off_ps, wo2_s[:, j, :], hid_sb[:, j, :],
                   start=(j == 0), stop=False)
            mm(off_ps, pmat_s, ref_t[:, s], start=False, stop=True)
            return m_sb, v_sb, off_ps, tz

        def stage2a(state):
            """Head-sum of q*k, grid-sample weight w, softmax partial sums."""
            m_sb, v_sb, off_ps, tz = state

            # qk head-sum one pipeline step after the GPSIMD q*k product so
            # the PE never waits on it
            qk_ps = psB.tile([32, tz], F32, tag="small")
            mm(qk_ps, amat_s[:, 0:32], m_sb[:, 0, :], start=True, stop=False)
            mm(qk_ps, amat_s[:, 32:64], m_sb[:, 1, :], start=False, stop=True)
            qk_sb = work.tile([32, tz], F32, tag="qks")
            nc.vector.tensor_copy(qk_sb, qk_ps)

            # w = relu(1-|sp_x-.5|)*relu(1-|sp_y-.5|); the y half is moved
            # to partitions 0-31 with a PE row-select matmul since DVE can't
            # pair operands at different base partitions
            t1_sb = work.tile([64, tz], F32, tag="t1")
            nc.scalar.activation(t1_sb, off_ps, AF.Abs, bias=bwof_s, scale=1.0)
            t2_sb = work.tile([64, tz], F32R, tag="t2")
            nc.scalar.activation(t2_sb, t1_sb, AF.Relu, bias=1.0, scale=-1.0)
            t2y_ps = psB.tile([32, tz], F32, tag="small")
            mm(t2y_ps, smat_s, t2_sb, start=True, stop=True)
            w_sb = work.tile([32, tz], F32, tag="w")
            nc.vector.tensor_mul(w_sb, t2_sb[0:32, :], t2y_ps)

            # softmax over K: e = exp(qk*w/sqrt(D))
            lg_sb = work.tile([32, tz], F32, tag="lg")
            nc.vector.tensor_mul(lg_sb, qk_sb, w_sb)
            e_sb = work.tile([32, tz], F32R, tag="e")
            nc.scalar.activation(e_sb, lg_sb, AF.Exp, bias=0.0, scale=SIGMA)
            ew_sb = work.tile([32, tz], F32R, tag="ew")
            nc.vector.tensor_mul(ew_sb, e_sb, w_sb)
            s1_ps = psB.tile([8, tz], F32, tag="small")
            mm(s1_ps, cmat_s, e_sb, start=True, stop=True)
            s2_ps = psB.tile([8, tz], F32, tag="small")
            mm(s2_ps, cmat_s, ew_sb, start=True, stop=True)
            return s1_ps, s2_ps, v_sb, tz

        def stage2b(state, g0):
            """Softmax normalization, ov = v*wv, out-projection, store."""
            s1_ps, s2_ps, v_sb, tz = state
            r1_sb = work.tile([8, tz], F32, tag="r1")
            nc.vector.reciprocal(r1_sb, s1_ps)
            wv_sb = work.tile([8, tz], F32R, tag="wvv")
            nc.vector.tensor_mul(wv_sb, s2_ps, r1_sb)

            # ov = v * wv (broadcast head->channels via matmul)
            ov_sb = work.tile([128, 2, tz], F32R, tag="ov")
            for mc in range(2):
                wvx_ps = psB.tile([128, tz], F32, tag="small")
                mm(wvx_ps, bmat_s[:, mc * 128:(mc + 1) * 128], wv_sb,
                   start=True, stop=True)
                nc.vector.tensor_mul(ov_sb[:, mc, :], v_sb[:, mc, :], wvx_ps)

            # out = ov.T @ Wout (+bout), token-major [T, 256]
            o_sb = work.tile([128, tz // 128, 256], F32, tag="osb")
            for q4 in range(tz // 128):
                o_ps = psB.tile([128, 256], F32, tag="small")
                q128 = slice(q4 * 128, (q4 + 1) * 128)
                mm(o_ps, ov_sb[:, 0, q128], wo_s[:, 0, :], start=True, stop=False)
                mm(o_ps, ov_sb[:, 1, q128], wo_s[:, 1, :], start=False,
                   stop=not with_bias)
                if with_bias:
                    mm(o_ps, ones_s[:, 0:128], bor_s, start=False, stop=True)
                nc.vector.tensor_copy(o_sb[:, q4, :], o_ps)
            nc.sync.dma_start(
                out=out[g0:g0 + tz, :].rearrange("(s2 p) c -> p s2 c", p=128),
                in_=o_sb)

        # 3-deep software pipeline: per iteration emit tile i's matmul-heavy
        # stage1, then tile i-2's output tail (stage2b), then tile i-1's
        # softmax chain (stage2a) — PE stays dense while ACT/DVE chains of
        # earlier tiles drain. stage2b(i-2) must precede stage2a(i-1) so the
        # s1/s2 PSUM slots recycle in trace order.
        assert nsub == 1
        # one full-width work unit per load tile (half-tile drain splitting
        # measured net-worse in the cost model: per-op overheads exceed the
        # drain savings)
        units = [(lt, 0, tload) for lt in range(nload)]
        p1 = p2 = None  # (state, g0) for stage2a / stage2b
        # first input tile before the weights so the PE can start ASAP;
        # weights ordered by first use
        wv_s = load1("wv", (128, 2, 256))
        ld = load_tile(0)
        wq_s = load1("wq", (128, 2, 256))
        wp_s = load1("wp", (128, 2, 256))
        wk_s = load1("wk", (128, 2, 256))
        wo1_s = load1("wo1", (128, 2, 512))
        bo1_s = load1("bo1", (128, 4))
        amat_s = load1("amat", (128, 64))
        wo2_s = load1("wo2", (128, 4, 64))
        pmat_s = load1("pmat", (2, 64))
        bwof_s = load1("bwof", (64, 1))
        smat_s = load1("smat", (64, 32))
        cmat_s = load1("cmat", (32, 8))
        bmat_s = load1("bmat", (8, 256))
        wo_s = load1("wo", (128, 2, 256))
        if with_bias:
            bqp_s = load1("bqp", (1, 256))
            bkp_s = load1("bkp", (1, 256))
            bvr_s = load1("bvr", (1, 256))
            bor_s = load1("bor", (1, 256))
            ones_s = load1("ones", (1, 512))
        ld_next = None
        cur_lt = 0
        for ui, (lt, lo, tz) in enumerate(units):
            if ui + 1 < len(units) and units[ui + 1][0] != lt:
                ld_next = load_tile(units[ui + 1][0])
            state = stage1(ld, lo, tz)
            if p2 is not None:
                stage2b(*p2)
                p2 = None
            if p1 is not None:
                st2, g0p = p1
                p2 = (stage2a(st2), g0p)
            p1 = (state, lt * tload + lo)
            if ui + 1 < len(units) and units[ui + 1][0] != lt:
                ld = ld_next
        if p2 is not None:
            stage2b(*p2)
        st2, g0p = p1
        stage2b(stage2a(st2), g0p)

    nc.compile()
    return nc


def _consts():
    amat = np.zeros((128, 64), np.float32)
    for mc in range(2):
        for d in range(128):
            h = mc * 4 + d // 32
            for k in range(KP):
                amat[d, mc * 32 + h * KP + k] = 1.0
    cmat = np.zeros((32, 8), np.float32)
    for j in range(32):
        cmat[j, j // KP] = 1.0
    bmat = np.zeros((8, 256), np.float32)
    for mc in range(2):
        for c in range(128):
            bmat[mc * 4 + c // 32, mc * 128 + c] = 1.0
    pmat = np.zeros((2, 64), np.float32)
    for r in range(64):
        pmat[r // 32, r] = 1.0
    smat = np.zeros((64, 32), np.float32)
    for j in range(32):
        smat[32 + j, j] = 1.0
    return amat, cmat, bmat, pmat, smat


def _wsplit(w):
    # [256, O] -> [128, 2, O]  (row kc*128+p  ->  [p, kc, :])
    o = w.shape[1]
    return np.ascontiguousarray(w.reshape(2, 128, o).transpose(1, 0, 2))


def _xsplit(x):
    # [T, 256] token-major -> [128, 2, T] channel-major chunks
    t = x.shape[0]
    return np.ascontiguousarray(x.T.reshape(2, 128, t).transpose(1, 0, 2))


def _host_maps(inputs, toks, ncores):
    f32 = lambda v: np.asarray(v, dtype=np.float32)
    query = f32(inputs["query"]).reshape(-1, C)
    key = f32(inputs["key"]).reshape(-1, C)
    value = f32(inputs["value"]).reshape(-1, C)
    pos = f32(inputs["pos_embed"]).reshape(-1, C)
    refp = f32(inputs["reference_points"]).reshape(-1, 2)

    # permute Wo2 columns (h,k,c) -> (c,h,k)
    perm = [h * (KP * 2) + k * 2 + c for c in range(2) for h in range(H)
            for k in range(KP)]
    wo2p = f32(inputs["Wo2"])[:, perm]
    bo2p = f32(inputs["bo2"])[perm]

    amat, cmat, bmat, pmat, smat = _consts()
    bqp = f32(inputs["bq"]) + f32(inputs["bpos"])
    bkp = f32(inputs["bk"]) + f32(inputs["bpos"])
    bv = f32(inputs["bv"])
    bout = f32(inputs["bout"])
    with_bias = any(np.any(b != 0) for b in (bqp, bkp, bv, bout))

    wo2r = np.ascontiguousarray(wo2p.reshape(4, 128, 64).transpose(1, 0, 2))
    shared = {
        "wq": _wsplit(f32(inputs["Wq"])),
        "wk": _wsplit(f32(inputs["Wk"])),
        "wv": _wsplit(f32(inputs["Wv"])),
        "wp": _wsplit(f32(inputs["Wpos"])),
        "wo1": _wsplit(f32(inputs["Wo1"])),
        "wo2": wo2r,
        "wo": _wsplit(f32(inputs["Wout"])),
        "bo1": np.ascontiguousarray(f32(inputs["bo1"]).reshape(4, 128).T),
        "bwof": np.ascontiguousarray((bo2p - 0.5).reshape(64, 1)),
        "smat": smat,
        "amat": amat, "cmat": cmat, "bmat": bmat, "pmat": pmat,
    }
    if with_bias:
        shared["ones"] = np.ones((1, 512), np.float32)
        shared["bqp"] = bqp.reshape(1, 256)
        shared["bkp"] = bkp.reshape(1, 256)
        shared["bvr"] = bv.reshape(1, 256)
        shared["bor"] = bout.reshape(1, 256)

    in_maps = []
    for cid in range(ncores):
        sl = slice(cid * toks, (cid + 1) * toks)
        m = dict(shared)
        m["xq"] = _xsplit(query[sl])
        m["xk"] = _xsplit(key[sl])
        m["xv"] = _xsplit(value[sl])
        m["xp"] = _xsplit(pos[sl])
        m["ref"] = np.ascontiguousarray(refp[sl].T)
        in_maps.append(m)
    return in_maps, with_bias


_NC_CACHE = {}


def kernel(**inputs):
    from concourse.bass_utils import run_bass_kernel_spmd

    in_maps, with_bias = _host_maps(inputs, TOKS, NCORES)
    ck = ("full", with_bias)
    if ck not in _NC_CACHE:
        _NC_CACHE[ck] = _build(toks=TOKS, tload=TLOAD, with_bias=with_bias)
    nc = _NC_CACHE[ck]
    res = run_bass_kernel_spmd(nc, in_maps, core_ids=list(range(NCORES)))
    outs = [r["out"] for r in res.results]
    full = np.concatenate(outs, axis=0).reshape(N, L, C)
    return np.ascontiguousarray(full.astype(np.float32))



# revision 45
# speedup vs baseline: 1.1732x; 1.1732x over previous
"""Trainium2 Bass kernel for ExtensibleAttention (sparse_attention).

Data-parallel over the 65536 tokens across 8 NeuronCores. Per 512-token
tile, all heavy projections (q, k, v, pos, MLP hidden) run as fp8e4m3
DoubleRow matmuls with a residual 3-pass (hi*hi + lo*hi + hi*lo) so each
256-deep contraction costs 1.5 PE rows instead of 4 (fp32) or 2 (bf16),
at ~bf16 accuracy. Activations are scaled x16 and weights x256 before
fp8 quantization to keep the residuals out of the e4m3 subnormal range;
the scales unfold for free into the existing ACT scale slots (hid relu),
the exp logit scale (qk carries S^2), and a host-side Wout/S.

The offset MLP second layer and the out-projection stay bf16. The
grid-sample weight, softmax over K=4, and head->channel broadcast reuse
small constant-matrix matmuls, with:
  - ref-points AND the bo2-0.5 bias folded into one [3,64] matmul
    (moving operand = [ref_x; ref_y; ones]),
  - channels host-permuted so head(p) is identical for both 128-channel
    chunks, making the wv head->channel broadcast a single matmul,
  - the t2 y-half moved to partition 0 by an SBUF->SBUF DMA instead of a
    PE row-select matmul,
  - softmax normalization as one DVE divide,
  - elementwise work spread across ACT / DVE / GPSIMD, and the per-tile
    work software-pipelined 3 deep with piece-wise emission so no engine
    head-of-line blocks the PE.
"""

import numpy as np
import ml_dtypes
from contextlib import ExitStack

import concourse.bacc as bacc
import concourse.tile as tile
from concourse import mybir

F32 = mybir.dt.float32
F32R = mybir.dt.float32r
F8 = mybir.dt.float8e4
BF16 = mybir.dt.bfloat16
F16 = mybir.dt.float16
AF = mybir.ActivationFunctionType
ALU = mybir.AluOpType
DRMODE = mybir.MatmulPerfMode.DoubleRow

E4 = ml_dtypes.float8_e4m3
BF = ml_dtypes.bfloat16

N, L, C, H, KP, D = 4, 16384, 256, 8, 4, 32
NCORES = 8
TOKS = N * L // NCORES  # 8192 tokens per core
T = 512                 # tokens per tile
SX = 16.0               # activation pre-scale for fp8
SW = 256.0              # weight pre-scale for fp8
S = SX * SW
SIGMA = float(1.0 / (np.sqrt(D) * S * S))   # exp scale absorbing S^2
INV_S = float(1.0 / S)


def _build(toks=TOKS, tload=T):
    nc = bacc.Bacc(trn_type="TRN2")

    xin = nc.dram_tensor("xin", [128, 16, toks], F8, kind="ExternalInput")
    w8 = nc.dram_tensor("w8", [128, 16, 256], F8, kind="ExternalInput")
    wo18 = nc.dram_tensor("wo18", [128, 4, 512], F8, kind="ExternalInput")
    wbf = nc.dram_tensor("wbf", [128, 800], BF16, kind="ExternalInput")
    wf32 = nc.dram_tensor("wf32", [128, 336], F32R, kind="ExternalInput")
    refo = nc.dram_tensor("refo", [3, toks], F32R, kind="ExternalInput")
    out = nc.dram_tensor("out", [toks, 256], F16, kind="ExternalOutput")

    ntiles = toks // tload
    global MM_LABELS
    MM_LABELS = []
    _mm_ctx = ["?"]

    _mm_n = [0]
    global MM_BY_NAME
    MM_BY_NAME = {}

    def mm(*a, **kw):
        lbl = f"{_mm_ctx[0]}#{_mm_n[0]}"
        MM_LABELS.append(lbl)
        _mm_n[0] += 1
        inst = nc.tensor.matmul(*a, **kw)
        try:
            MM_BY_NAME[inst.ins.name] = lbl
        except AttributeError:
            pass
        return inst

    with tile.TileContext(nc) as tc, ExitStack() as ctx:
        singles = ctx.enter_context(tc.tile_pool(name="singles", bufs=1))
        inp = ctx.enter_context(tc.tile_pool(name="inp", bufs=3))
        work = ctx.enter_context(tc.tile_pool(name="work", bufs=2))
        # PSUM budget (8 banks): big ring (v/q/k/hid) 4 + off 1 + qk 1 +
        # misc ring (wvx/out/s1/s2) 2
        ps = ctx.enter_context(tc.tile_pool(name="ps", bufs=1, space="PSUM"))

        def ps_tile(shape, tag, bufs):
            return ps.tile(shape, F32, tag=tag, bufs=bufs, name=f"ps_{tag}")

        # ---- weights / constants (one DMA each; sliced via APs) ----
        w8_s = singles.tile([128, 16, 256], F8, name="w8")
        nc.sync.dma_start(out=w8_s, in_=w8[:])
        ld0 = inp.tile([128, 16, tload], F8, tag="xin")
        # first tile split: v rows land first so the PE can start early
        nc.sync.dma_start(out=ld0[:, 8:12, :], in_=xin[:, 8:12, 0:tload])
        nc.sync.dma_start(out=ld0[:, 0:8, :], in_=xin[:, 0:8, 0:tload])
        nc.sync.dma_start(out=ld0[:, 12:16, :], in_=xin[:, 12:16, 0:tload])
        wo18_s = singles.tile([128, 4, 512], F8, name="wo18")
        nc.sync.dma_start(out=wo18_s, in_=wo18[:])
        wbf_s = singles.tile([128, 800], BF16, name="wbf")
        nc.sync.dma_start(out=wbf_s, in_=wbf[:])
        wf32_s = singles.tile([128, 336], F32R, name="wf32")
        nc.sync.dma_start(out=wf32_s, in_=wf32[:])
        refo_s = singles.tile([3, toks], F32R, name="refo")
        nc.sync.dma_start(out=refo_s, in_=refo[:])

        # stationary views
        # w8 rows: (t, hl, kc) with t in {q,k,v,p}: r = t*4 + hl*2 + kc
        def wpair(t, hl, mc):
            return w8_s[:, t * 4 + hl * 2:t * 4 + hl * 2 + 2,
                        mc * 128:(mc + 1) * 128]

        def wo1pair(hl, j):
            return wo18_s[:, hl * 2:hl * 2 + 2, j * 128:(j + 1) * 128]

        # wbf cols: wo2 [4*64 = 256] | wo [2*256 = 512] | amat [32]
        wo2_v = [wbf_s[:, j * 64:(j + 1) * 64] for j in range(4)]
        wo_v = [wbf_s[:, 256 + mc * 256:256 + (mc + 1) * 256]
                for mc in range(2)]
        amat_v = wbf_s[:, 768:800]
        # wf32 cols: cmat [8] | bmat [128] | pmat3 [64]
        cmatP_v = wf32_s[0:64, 0:16]
        bmatI_v = wf32_s[0:16, 16:144]
        bmatJ_v = wf32_s[0:16, 144:272]
        pmat3_v = wf32_s[0:3, 272:336]

        def xpair(ld, t, hl):
            return ld[:, t * 4 + hl * 2:t * 4 + hl * 2 + 2, :]

        def res3(ps, wt, ld, xt, start, stop=False):
            """3-pass fp8 DR contraction of input tensor xt with weight wt
            accumulated into ps (hi*hi, lo*hi, hi*lo)."""
            mm(ps, wpair(wt, 0, res3.mc), xpair(ld, xt, 0),
               start=start, stop=False, perf_mode=DRMODE)
            mm(ps, wpair(wt, 1, res3.mc), xpair(ld, xt, 0),
               start=False, stop=False, perf_mode=DRMODE)
            mm(ps, wpair(wt, 0, res3.mc), xpair(ld, xt, 1),
               start=False, stop=stop, perf_mode=DRMODE)

        st = {}   # per-tile pipeline state
        pst = {}  # per-pair pipeline state (key = even tile index)

        # PE clock warm-up: cheap matmuls on the (early-arriving) weights so
        # the p-state ramp completes during the first input-tile DMA
        _mm_ctx[0] = "warm"
        warm_ps = ps.tile([1, 64], F32, tag="misc", bufs=2, name="warm")
        for _ in range(24):
            mm(warm_ps, w8_s[:, 0, 0:1], w8_s[:, 0, 0:64],
               start=True, stop=True, skip_group_check=True)

        def emit_tile_front(i):
            """stage2a front for tile i: t1/t2/t2y-move/w (ACT, Pool, DMA)."""
            off_ps = st[i]["off"]
            t1_sb = work.tile([64, T], F32, tag="t1")
            nc.scalar.activation(t1_sb, off_ps, AF.Abs, bias=0.0, scale=1.0)
            t2_sb = work.tile([64, T], F32, tag="t2")
            nc.gpsimd.tensor_scalar(t2_sb, t1_sb, -1.0, 1.0,
                                    op0=ALU.mult, op1=ALU.add)
            nc.gpsimd.tensor_scalar_max(t2_sb, t2_sb, 0.0)
            t2y_sb = work.tile([32, T], F32, tag="t2y")
            nc.sync.dma_start(out=t2y_sb, in_=t2_sb[32:64, :])
            w_sb = work.tile([32, T], F32, tag="w")
            nc.gpsimd.tensor_mul(w_sb, t2_sb[0:32, :], t2y_sb)
            st[i]["w"] = w_sb

        def emit_v(i, ld):
            v_sb = work.tile([128, 2, T], BF16, tag="v", bufs=3)
            for mc in range(2):
                res3.mc = mc
                v_ps = ps_tile([128, T], "big", 4)
                res3(v_ps, 2, ld, 2, start=True, stop=True)
                nc.scalar.copy(v_sb[:, mc, :], v_ps)
            st[i]["v"] = v_sb

        def emit_bmat_ov(i):
            wvx_ps = ps_tile([128, T], "misc", 2)
            mm(wvx_ps, bmatI_v[0:8, :], st[i]["wv"], start=True, stop=True)
            ov_sb = work.tile([128, 2, T], BF16, tag="ov")
            v_sb = st[i]["v"]
            for mc in range(2):
                nc.vector.tensor_mul(ov_sb[:, mc, :], v_sb[:, mc, :], wvx_ps)
            st[i]["ov"] = ov_sb

        def emit_qk_chunk(i, ld, mc):
            if mc == 0:
                st[i]["m"] = work.tile([128, 2, T], BF16, tag="m", bufs=2,
                                       name="m_sb")
                st[i]["qsb"] = work.tile([128, 2, T], BF16, tag="qsb",
                                         bufs=2, name="q_sb")
            m_sb, q_sb = st[i]["m"], st[i]["qsb"]
            res3.mc = mc
            q_ps = ps_tile([128, T], "big", 4)
            res3(q_ps, 0, ld, 0, start=True)
            res3(q_ps, 3, ld, 3, start=False, stop=True)
            nc.scalar.copy(q_sb[:, mc, :], q_ps)
            k_ps = ps_tile([128, T], "big", 4)
            res3(k_ps, 1, ld, 1, start=True)
            res3(k_ps, 3, ld, 3, start=False, stop=True)
            nc.vector.tensor_mul(m_sb[:, mc, :], q_sb[:, mc, :], k_ps)

        def emit_hid(i, ld):
            hid_sb = work.tile([128, 4, T], BF16, tag="hid", bufs=2)
            for j in range(4):
                h_ps = ps_tile([128, T], "big", 4)
                mm(h_ps, wo1pair(0, j), xpair(ld, 0, 0),
                   start=True, stop=False, perf_mode=DRMODE)
                mm(h_ps, wo1pair(1, j), xpair(ld, 0, 0),
                   start=False, stop=False, perf_mode=DRMODE)
                mm(h_ps, wo1pair(0, j), xpair(ld, 0, 1),
                   start=False, stop=True, perf_mode=DRMODE)
                if j < 2:
                    nc.scalar.activation(hid_sb[:, j, :], h_ps, AF.Relu,
                                         bias=0.0, scale=INV_S)
                else:
                    nc.vector.tensor_scalar(hid_sb[:, j, :], h_ps,
                                            INV_S, 0.0,
                                            op0=ALU.mult, op1=ALU.max)
            st[i]["hid"] = hid_sb

        def emit_out(i, g0):
            ov_sb = st[i]["ov"]
            o_sb = work.tile([128, 4, 256], F16, tag="osb")
            for gp in range(2):
                o_ps = ps_tile([128, 2, 256], "misc", 2)
                for g4 in range(2):
                    g128 = slice((gp * 2 + g4) * 128,
                                 (gp * 2 + g4) * 128 + 128)
                    mm(o_ps[:, g4, :], ov_sb[:, 0, g128], wo_v[0],
                       start=True, stop=False)
                    mm(o_ps[:, g4, :], ov_sb[:, 1, g128], wo_v[1],
                       start=False, stop=True)
                nc.scalar.copy(o_sb[:, gp * 2:gp * 2 + 2, :], o_ps)
            nc.sync.dma_start(
                out=out[g0:g0 + T, :].rearrange("(g p) c -> p g c", p=128),
                in_=o_sb)

        def emit_off_amat(i, lt, ld):
            g0 = lt * T
            hid_sb = st[i]["hid"]
            off_ps = ps_tile([64, T], "off", 1)
            for j in range(4):
                mm(off_ps, wo2_v[j], hid_sb[:, j, :],
                   start=(j == 0), stop=False)
            mm(off_ps, pmat3_v, refo_s[:, g0:g0 + T], start=False, stop=True)
            m_sb = st[i]["m"]
            qk_ps = ps_tile([32, T], "qk", 1)
            mm(qk_ps, amat_v, m_sb[:, 0, :], start=True, stop=False)
            mm(qk_ps, amat_v, m_sb[:, 1, :], start=False, stop=True)
            st[i]["off"] = off_ps
            st[i]["qk"] = qk_ps

        def emit_tile_back(i):
            """stage2a back: lg, e, ew, cmat, reciprocal + mul."""
            w_sb = st[i]["w"]
            lg_sb = work.tile([32, T], F32, tag="lg")
            nc.vector.tensor_mul(lg_sb, st[i]["qk"], w_sb)
            e_sb = work.tile([32, T], F32R, tag="ee")
            nc.scalar.activation(e_sb, lg_sb, AF.Exp, bias=0.0, scale=SIGMA)
            ew_sb = work.tile([32, T], F32R, tag="ew")
            nc.gpsimd.tensor_mul(ew_sb, e_sb, w_sb)
            sA_ps = ps_tile([8, T], "misc", 2)
            mm(sA_ps, cmatP_v[0:32, 0:8], e_sb, start=True, stop=True)
            sB_ps = ps_tile([8, T], "misc", 2)
            mm(sB_ps, cmatP_v[0:32, 0:8], ew_sb, start=True, stop=True)
            rA_sb = work.tile([8, T], F32, tag="r1")
            nc.vector.reciprocal(rA_sb, sA_ps)
            wv_sb = work.tile([8, T], F32R, tag="wv")
            nc.vector.tensor_mul(wv_sb, sB_ps, rA_sb)
            st[i]["wv"] = wv_sb

        # ---- software pipeline, 3 deep, piece-wise emission ----
        ld = ld0
        for it in range(ntiles + 2):
            i, i1, i2 = it, it - 1, it - 2
            if i < ntiles:
                st[i] = {}
                if i + 1 < ntiles:
                    ld_next = inp.tile([128, 16, T], F8, tag="xin")
                    nc.sync.dma_start(
                        out=ld_next, in_=xin[:, :, (i + 1) * T:(i + 2) * T])
            if 0 <= i1 < ntiles:
                _mm_ctx[0] = f"front({i1})"
                emit_tile_front(i1)              # ACT t1, Pool t2, DMA, Pool w
            if i < ntiles:
                _mm_n[0] = 0; _mm_ctx[0] = f"v({i})"
                emit_v(i, ld)                    # PE v + ACT copies
            if i < ntiles:
                _mm_ctx[0] = f"qk({i})"
                emit_qk_chunk(i, ld, 0)          # PE q0,k0 + ACT copy + DVE m0
            if 0 <= i2 < ntiles:
                _mm_ctx[0] = f"bmat({i2})"
                emit_bmat_ov(i2)                 # PE bmat + DVE ov
            if i < ntiles:
                _mm_ctx[0] = f"qk({i})"
                emit_qk_chunk(i, ld, 1)          # PE q1,k1 + ACT copy + DVE m1
                _mm_ctx[0] = f"hid({i})"
                emit_hid(i, ld)                  # PE hid + ACT/DVE relu
            if 0 <= i2 < ntiles:
                _mm_ctx[0] = f"out({i2})"
                emit_out(i2, i2 * T)             # PE out + ACT copy + store
            if i < ntiles:
                _mm_ctx[0] = f"offamat({i})"
                emit_off_amat(i, i, ld)          # PE off + amat
            if 0 <= i1 < ntiles:
                _mm_ctx[0] = f"back({i1})"
                emit_tile_back(i1)               # DVE lg, ACT e, Pool ew,
                                                 # PE cmat, DVE recip+mul
            if 0 <= i2 < ntiles:
                st.pop(i2, None)
            if i + 1 < ntiles:
                ld = ld_next

    nc.compile()
    return nc


# ---------------- host-side data prep ----------------

def _q8(x):
    return np.asarray(x, np.float32).astype(E4)


def _hi_lo(x):
    x = np.asarray(x, np.float32)
    hi = _q8(x)
    lo = _q8(x - hi.astype(np.float32))
    return hi, lo


def _chan_perm():
    """new channel (kc*128 + p) <- old channel; head(p) = p//16 for both
    chunks: old c = h*32 + d -> kc = d//16, p = h*16 + d%16."""
    perm = np.empty(256, np.int64)
    for c in range(256):
        h, d = divmod(c, 32)
        kc, r = divmod(d, 16)
        perm[kc * 128 + h * 16 + r] = c
    return perm


def _pairs(a2d, cols):
    """[256, cols] -> [128, 2, cols] with channel kc*128+p at [p, kc]."""
    return np.ascontiguousarray(
        np.asarray(a2d).reshape(2, 128, cols).transpose(1, 0, 2))


def _host_maps(inputs, toks, ncores):
    f32 = lambda v: np.asarray(v, dtype=np.float32)
    query = f32(inputs["query"]).reshape(-1, C)
    key = f32(inputs["key"]).reshape(-1, C)
    value = f32(inputs["value"]).reshape(-1, C)
    pos = f32(inputs["pos_embed"]).reshape(-1, C)
    refp = f32(inputs["reference_points"]).reshape(-1, 2)

    for bname in ("bq", "bk", "bv", "bpos", "bo1"):
        assert not np.any(f32(inputs[bname])), f"nonzero {bname} unsupported"
    bout = f32(inputs["bout"])

    perm = _chan_perm()
    Wq = f32(inputs["Wq"])[:, perm] * SW
    Wk = f32(inputs["Wk"])[:, perm] * SW
    Wv = f32(inputs["Wv"])[:, perm] * SW
    Wp = f32(inputs["Wpos"])[:, perm] * SW
    Wo1 = f32(inputs["Wo1"]) * SW
    Wout = f32(inputs["Wout"])[perm, :] / S

    # Wo2 columns (h,k,c) -> (c,h,k)
    cperm = [h * (KP * 2) + k * 2 + c for c in range(2) for h in range(H)
             for k in range(KP)]
    wo2p = f32(inputs["Wo2"])[:, cperm]
    bwof = f32(inputs["bo2"])[cperm] - 0.5

    # w8: rows (t, hl, kc), cols = out channel (within chunk kc of inputs)
    w8 = np.zeros((128, 16, 256), E4)
    for t, W in enumerate((Wq, Wk, Wv, Wp)):
        for hl, a in enumerate(_hi_lo(W)):
            p = _pairs(a, 256)  # [128, 2(in-chunk), 256]
            w8[:, t * 4 + hl * 2 + 0, :] = p[:, 0, :]
            w8[:, t * 4 + hl * 2 + 1, :] = p[:, 1, :]
    wo18 = np.zeros((128, 4, 512), E4)
    for hl, a in enumerate(_hi_lo(Wo1)):
        p = _pairs(a, 512)
        wo18[:, hl * 2 + 0, :] = p[:, 0, :]
        wo18[:, hl * 2 + 1, :] = p[:, 1, :]

    # wbf: wo2 (4 chunks x 64) | wo (2 chunks x 256) | amat (32)
    amat = np.zeros((128, 32), np.float32)
    for p in range(128):
        h = p // 16
        amat[p, h * KP:(h + 1) * KP] = 1.0
    wbf = np.zeros((128, 800), BF)
    wo2r = np.ascontiguousarray(wo2p.reshape(4, 128, 64).transpose(1, 0, 2))
    wbf[:, 0:256] = wo2r.reshape(128, 256).astype(BF)
    wor = _pairs(Wout, 256)  # [128, 2, 256]
    wbf[:, 256:512] = wor[:, 0, :].astype(BF)
    wbf[:, 512:768] = wor[:, 1, :].astype(BF)
    wbf[:, 768:800] = amat.astype(BF)

    # wf32: cmatP [64x16] | bmatI [16x128] | bmatJ [16x128] | pmat3 [3x64]
    wf32 = np.zeros((128, 336), np.float32)
    cmatP = np.zeros((64, 16), np.float32)
    for r in range(64):
        cmatP[r, (r // 32) * 8 + (r % 32) // KP] = 1.0
    wf32[0:64, 0:16] = cmatP
    bmatI = np.zeros((16, 128), np.float32)
    bmatJ = np.zeros((16, 128), np.float32)
    for p in range(128):
        bmatI[p // 16, p] = 1.0
        bmatJ[8 + p // 16, p] = 1.0
    wf32[0:16, 16:144] = bmatI
    wf32[0:16, 144:272] = bmatJ
    pmat3 = np.zeros((3, 64), np.float32)
    pmat3[0, 0:32] = 1.0
    pmat3[1, 32:64] = 1.0
    pmat3[2, :] = bwof
    wf32[0:3, 272:336] = pmat3

    shared = {"w8": w8, "wo18": wo18, "wbf": wbf, "wf32": wf32}

    in_maps = []
    for cid in range(ncores):
        sl = slice(cid * toks, (cid + 1) * toks)
        xin = np.zeros((128, 16, toks), E4)
        for t, x in enumerate((query, key, value, pos)):
            for hl, a in enumerate(_hi_lo(x[sl].T * SX)):
                p = _pairs(a, toks)
                xin[:, t * 4 + hl * 2 + 0, :] = p[:, 0, :]
                xin[:, t * 4 + hl * 2 + 1, :] = p[:, 1, :]
        refo = np.ones((3, toks), np.float32)
        refo[0:2, :] = refp[sl].T
        m = dict(shared)
        m["xin"] = xin
        m["refo"] = refo
        in_maps.append(m)
    return in_maps, bout


_NC_CACHE = {}


def kernel(**inputs):
    from concourse.bass_utils import run_bass_kernel_spmd

    in_maps, bout = _host_maps(inputs, TOKS, NCORES)
    if "nc" not in _NC_CACHE:
        _NC_CACHE["nc"] = _build(toks=TOKS, tload=T)
    nc = _NC_CACHE["nc"]
    res = run_bass_kernel_spmd(nc, in_maps, core_ids=list(range(NCORES)))
    outs = [np.asarray(r["out"]).astype(np.float32) for r in res.results]
    full = np.concatenate(outs, axis=0).reshape(N, L, C) + bout
    return np.ascontiguousarray(full.astype(np.float32))


# revision 47
# speedup vs baseline: 1.2078x; 1.0294x over previous
"""Trainium2 Bass kernel for ExtensibleAttention (sparse_attention).

Data-parallel over the 65536 tokens across 8 NeuronCores. Per 512-token
tile, all heavy projections (q, k, v, pos, MLP hidden) run as fp8e4m3
DoubleRow matmuls with a residual 3-pass (hi*hi + lo*hi + hi*lo) so each
256-deep contraction costs 1.5 PE rows instead of 4 (fp32) or 2 (bf16),
at ~bf16 accuracy. Activations are scaled x16 and weights x256 before
fp8 quantization to keep the residuals out of the e4m3 subnormal range;
the scales unfold for free into the existing ACT scale slots (hid relu),
the exp logit scale (qk carries S^2), and a host-side Wout/S.

The offset MLP second layer and the out-projection stay bf16. The
grid-sample weight, softmax over K=4, and head->channel broadcast reuse
small constant-matrix matmuls, with:
  - ref-points AND the bo2-0.5 bias folded into one [3,64] matmul
    (moving operand = [ref_x; ref_y; ones]),
  - channels host-permuted so head(p) is identical for both 128-channel
    chunks, making the wv head->channel broadcast a single matmul,
  - the t2 y-half moved to partition 0 by an SBUF->SBUF DMA instead of a
    PE row-select matmul,
  - softmax normalization as one DVE divide,
  - elementwise work spread across ACT / DVE / GPSIMD, and the per-tile
    work software-pipelined 3 deep with piece-wise emission so no engine
    head-of-line blocks the PE.
"""

import numpy as np
import ml_dtypes
from contextlib import ExitStack

import concourse.bacc as bacc
import concourse.tile as tile
from concourse import mybir

F32 = mybir.dt.float32
F32R = mybir.dt.float32r
F8 = mybir.dt.float8e4
BF16 = mybir.dt.bfloat16
F16 = mybir.dt.float16
AF = mybir.ActivationFunctionType
ALU = mybir.AluOpType
DRMODE = mybir.MatmulPerfMode.DoubleRow

E4 = ml_dtypes.float8_e4m3
BF = ml_dtypes.bfloat16

N, L, C, H, KP, D = 4, 16384, 256, 8, 4, 32
NCORES = 8
TOKS = N * L // NCORES  # 8192 tokens per core
T = 512                 # tokens per tile
SX = 16.0               # activation pre-scale for fp8
SW = 256.0              # weight pre-scale for fp8
S = SX * SW
SIGMA = float(1.0 / (np.sqrt(D) * S * S))   # exp scale absorbing S^2
INV_S = float(1.0 / S)


def _build(toks=TOKS, tload=T):
    nc = bacc.Bacc(trn_type="TRN2")

    xin = nc.dram_tensor("xin", [128, 16, toks], F8, kind="ExternalInput")
    w8 = nc.dram_tensor("w8", [128, 16, 256], F8, kind="ExternalInput")
    wo18 = nc.dram_tensor("wo18", [128, 4, 512], F8, kind="ExternalInput")
    wbf = nc.dram_tensor("wbf", [128, 800], BF16, kind="ExternalInput")
    wf32 = nc.dram_tensor("wf32", [128, 336], F32R, kind="ExternalInput")
    refo = nc.dram_tensor("refo", [3, toks], F32R, kind="ExternalInput")
    out = nc.dram_tensor("out", [toks, 256], F16, kind="ExternalOutput")

    ntiles = toks // tload
    global MM_LABELS
    MM_LABELS = []
    _mm_ctx = ["?"]

    _mm_n = [0]
    global MM_BY_NAME
    MM_BY_NAME = {}

    def mm(*a, **kw):
        lbl = f"{_mm_ctx[0]}#{_mm_n[0]}"
        MM_LABELS.append(lbl)
        _mm_n[0] += 1
        inst = nc.tensor.matmul(*a, **kw)
        try:
            MM_BY_NAME[inst.ins.name] = lbl
        except AttributeError:
            pass
        return inst

    with tile.TileContext(nc) as tc, ExitStack() as ctx:
        singles = ctx.enter_context(tc.tile_pool(name="singles", bufs=1))
        inp = ctx.enter_context(tc.tile_pool(name="inp", bufs=3))
        work = ctx.enter_context(tc.tile_pool(name="work", bufs=2))
        # PSUM budget (8 banks): big ring (v/q/k/hid) 4 + off 1 + qk 1 +
        # misc ring (wvx/out/s1/s2) 2
        ps = ctx.enter_context(tc.tile_pool(name="ps", bufs=1, space="PSUM"))

        def ps_tile(shape, tag, bufs):
            return ps.tile(shape, F32, tag=tag, bufs=bufs, name=f"ps_{tag}")

        # ---- weights / constants (one DMA each; sliced via APs) ----
        w8_s = singles.tile([128, 16, 256], F8, name="w8")
        nc.sync.dma_start(out=w8_s[:, 0:1, :], in_=w8[:, 0:1, :])
        nc.sync.dma_start(out=w8_s[:, 1:16, :], in_=w8[:, 1:16, :])
        ld0 = inp.tile([128, 16, tload], F8, tag="xin")
        # first tile split: v rows land first so the PE can start early
        nc.sync.dma_start(out=ld0[:, 8:12, :], in_=xin[:, 8:12, 0:tload])
        nc.sync.dma_start(out=ld0[:, 0:8, :], in_=xin[:, 0:8, 0:tload])
        nc.sync.dma_start(out=ld0[:, 12:16, :], in_=xin[:, 12:16, 0:tload])
        wo18_s = singles.tile([128, 4, 512], F8, name="wo18")
        nc.sync.dma_start(out=wo18_s, in_=wo18[:])
        wbf_s = singles.tile([128, 800], BF16, name="wbf")
        nc.sync.dma_start(out=wbf_s, in_=wbf[:])
        wf32_s = singles.tile([128, 336], F32R, name="wf32")
        nc.sync.dma_start(out=wf32_s, in_=wf32[:])
        refo_s = singles.tile([3, toks], F32R, name="refo")
        nc.sync.dma_start(out=refo_s, in_=refo[:])

        # stationary views
        # w8 rows: (t, hl, kc) with t in {q,k,v,p}: r = t*4 + hl*2 + kc
        def wpair(t, hl, mc):
            return w8_s[:, t * 4 + hl * 2:t * 4 + hl * 2 + 2,
                        mc * 128:(mc + 1) * 128]

        def wo1pair(hl, j):
            return wo18_s[:, hl * 2:hl * 2 + 2, j * 128:(j + 1) * 128]

        # wbf cols: wo2 [4*64 = 256] | wo [2*256 = 512] | amat [32]
        wo2_v = [wbf_s[:, j * 64:(j + 1) * 64] for j in range(4)]
        wo_v = [wbf_s[:, 256 + mc * 256:256 + (mc + 1) * 256]
                for mc in range(2)]
        amat_v = wbf_s[:, 768:800]
        # wf32 cols: cmat [8] | bmat [128] | pmat3 [64]
        cmatP_v = wf32_s[0:64, 0:16]
        bmatI_v = wf32_s[0:16, 16:144]
        bmatJ_v = wf32_s[0:16, 144:272]
        pmat3_v = wf32_s[0:3, 272:336]

        def xpair(ld, t, hl):
            return ld[:, t * 4 + hl * 2:t * 4 + hl * 2 + 2, :]

        def res3(ps, wt, ld, xt, start, stop=False):
            """3-pass fp8 DR contraction of input tensor xt with weight wt
            accumulated into ps (hi*hi, lo*hi, hi*lo)."""
            mm(ps, wpair(wt, 0, res3.mc), xpair(ld, xt, 0),
               start=start, stop=False, perf_mode=DRMODE)
            mm(ps, wpair(wt, 1, res3.mc), xpair(ld, xt, 0),
               start=False, stop=False, perf_mode=DRMODE)
            mm(ps, wpair(wt, 0, res3.mc), xpair(ld, xt, 1),
               start=False, stop=stop, perf_mode=DRMODE)

        st = {}   # per-tile pipeline state
        pst = {}  # per-pair pipeline state (key = even tile index)

        # PE clock warm-up: cheap matmuls on the (early-arriving) weights so
        # the p-state ramp completes during the first input-tile DMA
        _mm_ctx[0] = "warm"
        warm_ps = ps.tile([1, 64], F32, tag="misc", bufs=2, name="warm")
        for _ in range(24):
            mm(warm_ps, w8_s[:, 0, 0:1], w8_s[:, 0, 0:64],
               start=True, stop=True, skip_group_check=True)

        def emit_tile_front(i):
            """stage2a front for tile i: t1/t2/t2y-move/w (ACT, Pool, DMA)."""
            off_ps = st[i]["off"]
            t1_sb = work.tile([64, T], F32, tag="t1")
            nc.scalar.activation(t1_sb, off_ps, AF.Abs, bias=0.0, scale=1.0)
            t2_sb = work.tile([64, T], F32, tag="t2")
            nc.scalar.activation(t2_sb, t1_sb, AF.Relu, bias=1.0, scale=-1.0)
            t2y_sb = work.tile([32, T], F32, tag="t2y")
            nc.sync.dma_start(out=t2y_sb, in_=t2_sb[32:64, :])
            w_sb = work.tile([32, T], F32, tag="w")
            nc.gpsimd.tensor_mul(w_sb, t2_sb[0:32, :], t2y_sb)
            st[i]["w"] = w_sb

        def emit_v(i, ld):
            v_sb = work.tile([128, 2, T], BF16, tag="v", bufs=3)
            for mc in range(2):
                res3.mc = mc
                v_ps = ps_tile([128, T], "big", 4)
                res3(v_ps, 2, ld, 2, start=True, stop=True)
                nc.scalar.copy(v_sb[:, mc, :], v_ps)
            st[i]["v"] = v_sb

        def emit_bmat_ov(i):
            wvx_ps = ps_tile([128, T], "misc", 2)
            mm(wvx_ps, bmatI_v[0:8, :], st[i]["wv"], start=True, stop=True)
            ov_sb = work.tile([128, 2, T], BF16, tag="ov")
            v_sb = st[i]["v"]
            for mc in range(2):
                nc.vector.tensor_mul(ov_sb[:, mc, :], v_sb[:, mc, :], wvx_ps)
            st[i]["ov"] = ov_sb

        def emit_qk_chunk(i, ld, mc):
            if mc == 0:
                st[i]["m"] = work.tile([128, 2, T], BF16, tag="m", bufs=2,
                                       name="m_sb")
                st[i]["qsb"] = work.tile([128, 2, T], BF16, tag="qsb",
                                         bufs=2, name="q_sb")
            m_sb, q_sb = st[i]["m"], st[i]["qsb"]
            res3.mc = mc
            q_ps = ps_tile([128, T], "big", 4)
            res3(q_ps, 0, ld, 0, start=True)
            res3(q_ps, 3, ld, 3, start=False, stop=True)
            nc.scalar.copy(q_sb[:, mc, :], q_ps)
            k_ps = ps_tile([128, T], "big", 4)
            res3(k_ps, 1, ld, 1, start=True)
            res3(k_ps, 3, ld, 3, start=False, stop=True)
            nc.vector.tensor_mul(m_sb[:, mc, :], q_sb[:, mc, :], k_ps)

        def emit_hid(i, ld):
            hid_sb = work.tile([128, 4, T], BF16, tag="hid", bufs=2)
            for j in range(4):
                h_ps = ps_tile([128, T], "big", 4)
                mm(h_ps, wo1pair(0, j), xpair(ld, 0, 0),
                   start=True, stop=False, perf_mode=DRMODE)
                mm(h_ps, wo1pair(1, j), xpair(ld, 0, 0),
                   start=False, stop=False, perf_mode=DRMODE)
                mm(h_ps, wo1pair(0, j), xpair(ld, 0, 1),
                   start=False, stop=True, perf_mode=DRMODE)
                if j < 2:
                    nc.scalar.activation(hid_sb[:, j, :], h_ps, AF.Relu,
                                         bias=0.0, scale=INV_S)
                else:
                    nc.vector.tensor_scalar(hid_sb[:, j, :], h_ps,
                                            INV_S, 0.0,
                                            op0=ALU.mult, op1=ALU.max)
            st[i]["hid"] = hid_sb

        def emit_out(i, g0):
            ov_sb = st[i]["ov"]
            o_sb = work.tile([128, 4, 256], F16, tag="osb")
            for gp in range(2):
                o_ps = ps_tile([128, 2, 256], "misc", 2)
                for g4 in range(2):
                    g128 = slice((gp * 2 + g4) * 128,
                                 (gp * 2 + g4) * 128 + 128)
                    mm(o_ps[:, g4, :], ov_sb[:, 0, g128], wo_v[0],
                       start=True, stop=False)
                    mm(o_ps[:, g4, :], ov_sb[:, 1, g128], wo_v[1],
                       start=False, stop=True)
                nc.scalar.copy(o_sb[:, gp * 2:gp * 2 + 2, :], o_ps)
            nc.sync.dma_start(
                out=out[g0:g0 + T, :].rearrange("(g p) c -> p g c", p=128),
                in_=o_sb)

        def emit_off_amat(i, lt, ld):
            g0 = lt * T
            hid_sb = st[i]["hid"]
            off_ps = ps_tile([64, T], "off", 1)
            for j in range(4):
                mm(off_ps, wo2_v[j], hid_sb[:, j, :],
                   start=(j == 0), stop=False)
            mm(off_ps, pmat3_v, refo_s[:, g0:g0 + T], start=False, stop=True)
            m_sb = st[i]["m"]
            qk_ps = ps_tile([32, T], "qk", 1)
            mm(qk_ps, amat_v, m_sb[:, 0, :], start=True, stop=False)
            mm(qk_ps, amat_v, m_sb[:, 1, :], start=False, stop=True)
            st[i]["off"] = off_ps
            st[i]["qk"] = qk_ps

        def emit_tile_back(i):
            """stage2a back: lg, e, ew, cmat, reciprocal + mul."""
            w_sb = st[i]["w"]
            lg_sb = work.tile([32, T], F32, tag="lg")
            nc.vector.tensor_mul(lg_sb, st[i]["qk"], w_sb)
            e_sb = work.tile([32, T], F32R, tag="ee")
            nc.scalar.activation(e_sb, lg_sb, AF.Exp, bias=0.0, scale=SIGMA)
            ew_sb = work.tile([32, T], F32R, tag="ew")
            nc.gpsimd.tensor_mul(ew_sb, e_sb, w_sb)
            sA_ps = ps_tile([8, T], "misc", 2)
            mm(sA_ps, cmatP_v[0:32, 0:8], e_sb, start=True, stop=True)
            sB_ps = ps_tile([8, T], "misc", 2)
            mm(sB_ps, cmatP_v[0:32, 0:8], ew_sb, start=True, stop=True)
            rA_sb = work.tile([8, T], F32, tag="r1")
            nc.vector.reciprocal(rA_sb, sA_ps)
            wv_sb = work.tile([8, T], F32R, tag="wv")
            nc.vector.tensor_mul(wv_sb, sB_ps, rA_sb)
            st[i]["wv"] = wv_sb

        # ---- software pipeline, 3 deep, piece-wise emission ----
        ld = ld0
        for it in range(ntiles + 2):
            i, i1, i2 = it, it - 1, it - 2
            if i < ntiles:
                st[i] = {}
                if i + 1 < ntiles:
                    ld_next = inp.tile([128, 16, T], F8, tag="xin")
                    nc.sync.dma_start(
                        out=ld_next, in_=xin[:, :, (i + 1) * T:(i + 2) * T])
            if 0 <= i1 < ntiles:
                _mm_ctx[0] = f"front({i1})"
                emit_tile_front(i1)              # ACT t1, Pool t2, DMA, Pool w
            if i < ntiles:
                _mm_n[0] = 0; _mm_ctx[0] = f"v({i})"
                emit_v(i, ld)                    # PE v + ACT copies
            if i < ntiles:
                _mm_ctx[0] = f"qk({i})"
                emit_qk_chunk(i, ld, 0)          # PE q0,k0 + ACT copy + DVE m0
            if 0 <= i2 < ntiles:
                _mm_ctx[0] = f"bmat({i2})"
                emit_bmat_ov(i2)                 # PE bmat + DVE ov
            if i < ntiles:
                _mm_ctx[0] = f"qk({i})"
                emit_qk_chunk(i, ld, 1)          # PE q1,k1 + ACT copy + DVE m1
                _mm_ctx[0] = f"hid({i})"
                emit_hid(i, ld)                  # PE hid + ACT/DVE relu
            if 0 <= i2 < ntiles:
                _mm_ctx[0] = f"out({i2})"
                emit_out(i2, i2 * T)             # PE out + ACT copy + store
            if i < ntiles:
                _mm_ctx[0] = f"offamat({i})"
                emit_off_amat(i, i, ld)          # PE off + amat
            if 0 <= i1 < ntiles:
                _mm_ctx[0] = f"back({i1})"
                emit_tile_back(i1)               # DVE lg, ACT e, Pool ew,
                                                 # PE cmat, DVE recip+mul
            if 0 <= i2 < ntiles:
                st.pop(i2, None)
            if i + 1 < ntiles:
                ld = ld_next

    nc.compile()
    return nc


# ---------------- host-side data prep ----------------

def _q8(x):
    return np.asarray(x, np.float32).astype(E4)


def _hi_lo(x):
    x = np.asarray(x, np.float32)
    hi = _q8(x)
    lo = _q8(x - hi.astype(np.float32))
    return hi, lo


def _chan_perm():
    """new channel (kc*128 + p) <- old channel; head(p) = p//16 for both
    chunks: old c = h*32 + d -> kc = d//16, p = h*16 + d%16."""
    perm = np.empty(256, np.int64)
    for c in range(256):
        h, d = divmod(c, 32)
        kc, r = divmod(d, 16)
        perm[kc * 128 + h * 16 + r] = c
    return perm


def _pairs(a2d, cols):
    """[256, cols] -> [128, 2, cols] with channel kc*128+p at [p, kc]."""
    return np.ascontiguousarray(
        np.asarray(a2d).reshape(2, 128, cols).transpose(1, 0, 2))


def _host_maps(inputs, toks, ncores):
    f32 = lambda v: np.asarray(v, dtype=np.float32)
    query = f32(inputs["query"]).reshape(-1, C)
    key = f32(inputs["key"]).reshape(-1, C)
    value = f32(inputs["value"]).reshape(-1, C)
    pos = f32(inputs["pos_embed"]).reshape(-1, C)
    refp = f32(inputs["reference_points"]).reshape(-1, 2)

    for bname in ("bq", "bk", "bv", "bpos", "bo1"):
        assert not np.any(f32(inputs[bname])), f"nonzero {bname} unsupported"
    bout = f32(inputs["bout"])

    perm = _chan_perm()
    Wq = f32(inputs["Wq"])[:, perm] * SW
    Wk = f32(inputs["Wk"])[:, perm] * SW
    Wv = f32(inputs["Wv"])[:, perm] * SW
    Wp = f32(inputs["Wpos"])[:, perm] * SW
    Wo1 = f32(inputs["Wo1"]) * SW
    Wout = f32(inputs["Wout"])[perm, :] / S

    # Wo2 columns (h,k,c) -> (c,h,k)
    cperm = [h * (KP * 2) + k * 2 + c for c in range(2) for h in range(H)
             for k in range(KP)]
    wo2p = f32(inputs["Wo2"])[:, cperm]
    bwof = f32(inputs["bo2"])[cperm] - 0.5

    # w8: rows (t, hl, kc), cols = out channel (within chunk kc of inputs)
    w8 = np.zeros((128, 16, 256), E4)
    for t, W in enumerate((Wq, Wk, Wv, Wp)):
        for hl, a in enumerate(_hi_lo(W)):
            p = _pairs(a, 256)  # [128, 2(in-chunk), 256]
            w8[:, t * 4 + hl * 2 + 0, :] = p[:, 0, :]
            w8[:, t * 4 + hl * 2 + 1, :] = p[:, 1, :]
    wo18 = np.zeros((128, 4, 512), E4)
    for hl, a in enumerate(_hi_lo(Wo1)):
        p = _pairs(a, 512)
        wo18[:, hl * 2 + 0, :] = p[:, 0, :]
        wo18[:, hl * 2 + 1, :] = p[:, 1, :]

    # wbf: wo2 (4 chunks x 64) | wo (2 chunks x 256) | amat (32)
    amat = np.zeros((128, 32), np.float32)
    for p in range(128):
        h = p // 16
        amat[p, h * KP:(h + 1) * KP] = 1.0
    wbf = np.zeros((128, 800), BF)
    wo2r = np.ascontiguousarray(wo2p.reshape(4, 128, 64).transpose(1, 0, 2))
    wbf[:, 0:256] = wo2r.reshape(128, 256).astype(BF)
    wor = _pairs(Wout, 256)  # [128, 2, 256]
    wbf[:, 256:512] = wor[:, 0, :].astype(BF)
    wbf[:, 512:768] = wor[:, 1, :].astype(BF)
    wbf[:, 768:800] = amat.astype(BF)

    # wf32: cmatP [64x16] | bmatI [16x128] | bmatJ [16x128] | pmat3 [3x64]
    wf32 = np.zeros((128, 336), np.float32)
    cmatP = np.zeros((64, 16), np.float32)
    for r in range(64):
        cmatP[r, (r // 32) * 8 + (r % 32) // KP] = 1.0
    wf32[0:64, 0:16] = cmatP
    bmatI = np.zeros((16, 128), np.float32)
    bmatJ = np.zeros((16, 128), np.float32)
    for p in range(128):
        bmatI[p // 16, p] = 1.0
        bmatJ[8 + p // 16, p] = 1.0
    wf32[0:16, 16:144] = bmatI
    wf32[0:16, 144:272] = bmatJ
    pmat3 = np.zeros((3, 64), np.float32)
    pmat3[0, 0:32] = 1.0
    pmat3[1, 32:64] = 1.0
    pmat3[2, :] = bwof
    wf32[0:3, 272:336] = pmat3

    shared = {"w8": w8, "wo18": wo18, "wbf": wbf, "wf32": wf32}

    in_maps = []
    for cid in range(ncores):
        sl = slice(cid * toks, (cid + 1) * toks)
        xin = np.zeros((128, 16, toks), E4)
        for t, x in enumerate((query, key, value, pos)):
            for hl, a in enumerate(_hi_lo(x[sl].T * SX)):
                p = _pairs(a, toks)
                xin[:, t * 4 + hl * 2 + 0, :] = p[:, 0, :]
                xin[:, t * 4 + hl * 2 + 1, :] = p[:, 1, :]
        refo = np.ones((3, toks), np.float32)
        refo[0:2, :] = refp[sl].T
        m = dict(shared)
        m["xin"] = xin
        m["refo"] = refo
        in_maps.append(m)
    return in_maps, bout


_NC_CACHE = {}


def kernel(**inputs):
    from concourse.bass_utils import run_bass_kernel_spmd

    in_maps, bout = _host_maps(inputs, TOKS, NCORES)
    if "nc" not in _NC_CACHE:
        _NC_CACHE["nc"] = _build(toks=TOKS, tload=T)
    nc = _NC_CACHE["nc"]
    res = run_bass_kernel_spmd(nc, in_maps, core_ids=list(range(NCORES)))
    outs = [np.asarray(r["out"]).astype(np.float32) for r in res.results]
    full = np.concatenate(outs, axis=0).reshape(N, L, C) + bout
    return np.ascontiguousarray(full.astype(np.float32))
